# revision 24
# baseline (speedup 1.0000x reference)
"""AxonalConnections GNN message passing on 8 TRN2 NeuronCores.

out[n] = sum_{e: dst[e]==n} spikes[src[e]] * masks[src[e]] * weights[e]

Sharding: H (1024) split across 8 cores -> 128 h-rows per core, pure data
parallel (edges replicated), no collectives.

Per-core layout: partition p = s*16 + hh  (s = source node 0..7,
hh = h-block 0..15, each block 8 h-rows), free dims = (b, h''(8), f).
All inputs staged host-side in fp16, chunk-contiguous, so each W-chunk is
ONE big DMA (9KB/partition descriptor).

Engines:
  Pool (GpSimd): mod[s,b] = spikes[s,b] * masks[s]
  DVE:           sig[s,k,b] = mod[s,b] * w[s,k]   (fp16 packed -> 2x mode)
  PE:            out[n] = sum_k P_k @ sig[:,k]    (4 accumulating matmuls,
                 P_k = constant 0/1 edge-routing matrix, contraction over
                 the (s,hh) partition dim; PSUM fp32 accumulate)
  ACT:           PSUM -> SBUF fp16 copy + output DMA queue (HWDGE)
  SP:            input DMA queue (HWDGE)
"""

import numpy as np

import concourse.bacc as bacc
import concourse.mybir as mybir
import concourse.tile as tile
from concourse.bass_utils import run_bass_kernel_spmd

# Problem shape (hardcoded per spec)
N_NODES = 8
N_EDGES = 32
KDEG = 4            # out-edges per source node
B = 4
H = 1024
W = 1024
N_CORES = 8
H_SH = H // N_CORES          # 128 h-rows per core
HH = 16                      # h-blocks per core (partition sub-index)
HB = H_SH // HH              # 8 h-rows per block (free dim)
F = 64                       # w-chunk size
N_CHUNK = W // F             # 16

SP_F = B * HB * F            # 2048 spikes elems / partition / chunk
W_F = KDEG * HB * F          # 2048 weight elems
MK_F = HB * F                # 512 mask elems
IN_F = SP_F + W_F + MK_F     # 4608

F16 = mybir.dt.float16
F32 = mybir.dt.float32


def _edge_table(src, dst):
    """Group edges by source: returns (edge_ids[s][k], dst_of[s][k])."""
    eids = [[] for _ in range(N_NODES)]
    for e in range(N_EDGES):
        eids[src[e]].append(e)
    assert all(len(x) == KDEG for x in eids), "need exactly 4 out-edges per node"
    dsts = [[dst[e] for e in eids[s]] for s in range(N_NODES)]
    return eids, dsts


def _build_program(nc, src, dst):
    in_d = nc.dram_tensor("inbuf", [N_CHUNK, 128, IN_F], F16, kind="ExternalInput").ap()
    wm_d = nc.dram_tensor("wmat", [128, KDEG, 128], F16, kind="ExternalInput").ap()
    out_d = nc.dram_tensor("out", [N_CHUNK, 128, SP_F], F16, kind="ExternalOutput").ap()

    with tile.TileContext(nc) as tc:
        with (
            tc.tile_pool(name="in", bufs=4) as in_pool,
            tc.tile_pool(name="wm", bufs=1) as wm_pool,
            tc.tile_pool(name="mod", bufs=2) as mod_pool,
            tc.tile_pool(name="sig", bufs=4) as sig_pool,
            tc.psum_pool(name="ps", bufs=2) as ps_pool,
            tc.tile_pool(name="outs", bufs=3) as out_pool,
        ):
            wm_t = wm_pool.tile([128, KDEG, 128], F16)
            nc.scalar.dma_start(out=wm_t[:], in_=wm_d)

            HF = HB * F  # 512: contiguous inner (h'', f) span
            MM = 512     # max moving cols per matmul (hard ISA limit)
            for c in range(N_CHUNK):
                it = in_pool.tile([128, IN_F], F16, tag="in")
                if c == 0:
                    # first chunk: spikes+masks on the SP queue while the
                    # weights ride the otherwise-idle ACT queue in parallel
                    nc.sync.dma_start(out=it[:, 0:SP_F], in_=in_d[c][:, 0:SP_F])
                    nc.sync.dma_start(
                        out=it[:, SP_F + W_F : IN_F],
                        in_=in_d[c][:, SP_F + W_F : IN_F],
                    )
                    nc.scalar.dma_start(
                        out=it[:, SP_F : SP_F + W_F],
                        in_=in_d[c][:, SP_F : SP_F + W_F],
                    )
                else:
                    nc.sync.dma_start(out=it[:], in_=in_d[c])
                sp_v = it[:, 0:SP_F].rearrange("p (b q) -> p b q", b=B)
                w_v = it[:, SP_F : SP_F + W_F].rearrange("p (k q) -> p k q", k=KDEG)
                mk_v = it[:, SP_F + W_F : IN_F]

                # inner dim 512 keeps the DVE 2x (16-bit packed) mode engaged
                # (NOTE: GpSimd is useless here — its SBUF traffic slows
                # concurrent DVE ops ~3x)
                mod_t = mod_pool.tile([128, B, HF], F16, tag="mod")
                sig_t = sig_pool.tile([128, KDEG, B, HF], F16, tag="sig")
                ps = ps_pool.tile([128, SP_F], F32, tag="ps")
                out_s = out_pool.tile([128, SP_F], F16, tag="outs")

                if c == N_CHUNK - 1:
                    # drain the tail on half-sized pieces: process the last
                    # chunk in two b-half pipelines so the final PE/copy/DMA
                    # chain follows a half-sized sig
                    for bh in range(2):
                        bsl = slice(bh * 2, bh * 2 + 2)
                        qsl = slice(bh * (SP_F // 2), (bh + 1) * (SP_F // 2))
                        nc.vector.tensor_mul(
                            out=mod_t[:, bsl],
                            in0=sp_v[:, bsl],
                            in1=mk_v[:, None].broadcast_to([128, 2, HF]),
                        )
                        for half in range(2):
                            ksl = slice(half * 2, half * 2 + 2)
                            nc.vector.tensor_mul(
                                out=sig_t[:, ksl, bsl],
                                in0=mod_t[:, bsl][:, None].broadcast_to(
                                    [128, 2, 2, HF]
                                ),
                                in1=w_v[:, ksl, None].broadcast_to(
                                    [128, 2, 2, HF]
                                ),
                            )
                            for kk in range(2):
                                k = half * 2 + kk
                                sig_k = sig_t[:, k, bsl].rearrange(
                                    "p b q -> p (b q)"
                                )
                                for m in range(0, SP_F // 2, MM):
                                    nc.tensor.matmul(
                                        out=ps[:, bh * (SP_F // 2) + m :
                                               bh * (SP_F // 2) + m + MM],
                                        lhsT=wm_t[:, k],
                                        rhs=sig_k[:, m : m + MM],
                                        start=(k == 0),
                                        stop=(k == KDEG - 1),
                                        skip_group_check=True,
                                    )
                        nc.scalar.copy(out=out_s[:, qsl], in_=ps[:, qsl])
                        nc.scalar.dma_start(
                            out=out_d[c][:, qsl], in_=out_s[:, qsl]
                        )
                    continue

                nc.vector.tensor_mul(
                    out=mod_t[:],
                    in0=sp_v,
                    in1=mk_v[:, None].broadcast_to([128, B, HF]),
                )

                # split sig by k-halves so PE can start on the first half
                # while the DVE computes the second (one shared tile: the
                # WAR hazard throttles PE/DVE overlap, which measures FASTER
                # than full overlap — SBUF port contention costs more)
                for half in range(2):
                    ksl = slice(half * 2, half * 2 + 2)
                    nc.vector.tensor_mul(
                        out=sig_t[:, ksl],
                        in0=mod_t[:, None].broadcast_to([128, 2, B, HF]),
                        in1=w_v[:, ksl, None].broadcast_to([128, 2, B, HF]),
                    )
                    for kk in range(2):
                        k = half * 2 + kk
                        sig_k = sig_t[:, k].rearrange("p b q -> p (b q)")
                        for m in range(0, SP_F, MM):
                            nc.tensor.matmul(
                                out=ps[:, m : m + MM],
                                lhsT=wm_t[:, k],
                                rhs=sig_k[:, m : m + MM],
                                start=(k == 0),
                                stop=(k == KDEG - 1),
                                skip_group_check=True,
                            )

                nc.scalar.copy(out=out_s[:], in_=ps[:])
                nc.scalar.dma_start(out=out_d[c], in_=out_s[:])
    return out_d


def _trace_and_compile(src, dst):
    nc = bacc.Bacc(
        "TRN2",
        target_bir_lowering=False,
        debug=False,
        num_devices=N_CORES,
    )
    _build_program(nc, src, dst)
    nc.compile()
    return nc


def make_in_maps(spikes, masks, weights, src, dst):
    """Stage fp16 chunk-contiguous per-core input buffers."""
    eids, dsts = _edge_table(src, dst)

    # wmat[p_in = s*HH+hh, k, p_out = n*HH+hh] = 1 iff dst(s,k) == n
    wmat = np.zeros((128, KDEG, 128), dtype=np.float16)
    for s in range(N_NODES):
        for k in range(KDEG):
            n = dsts[s][k]
            for hh in range(HH):
                wmat[s * HH + hh, k, n * HH + hh] = 1.0

    # weights sorted into (s, k) slot order
    perm = [eids[s][k] for s in range(N_NODES) for k in range(KDEG)]
    w_sorted = weights[perm].astype(np.float16)  # [32, H, W]
    spikes16 = spikes.astype(np.float16)
    masks16 = masks.astype(np.float16)

    in_maps = []
    for i in range(N_CORES):
        hsl = slice(i * H_SH, (i + 1) * H_SH)
        # spikes [S,B,H_SH,W] -> [C, (s,hh), (b,h'',f)]
        sp = (
            spikes16[:, :, hsl, :]
            .reshape(N_NODES, B, HH, HB, N_CHUNK, F)
            .transpose(4, 0, 2, 1, 3, 5)
            .reshape(N_CHUNK, 128, SP_F)
        )
        # weights [32,H_SH,W] -> [C, (s,hh), (k,h'',f)]
        wv = (
            w_sorted[:, hsl, :]
            .reshape(N_NODES, KDEG, HH, HB, N_CHUNK, F)
            .transpose(4, 0, 2, 1, 3, 5)
            .reshape(N_CHUNK, 128, W_F)
        )
        # masks [S,H_SH,W] -> [C, (s,hh), (h'',f)]
        mk = (
            masks16[:, hsl, :]
            .reshape(N_NODES, HH, HB, N_CHUNK, F)
            .transpose(3, 0, 1, 2, 4)
            .reshape(N_CHUNK, 128, MK_F)
        )
        inbuf = np.concatenate([sp, wv, mk], axis=2)
        in_maps.append(
            {
                "inbuf": np.ascontiguousarray(inbuf),
                "wmat": wmat,
            }
        )
    return in_maps


def assemble_out(results):
    """[C, (n,hh), (b,h'',f)] fp16 per core -> [N, B, H, W] fp32."""
    out = np.empty((N_NODES, B, H, W), dtype=np.float32)
    for i in range(N_CORES):
        o = np.asarray(results[i]["out"], dtype=np.float32)
        o = (
            o.reshape(N_CHUNK, N_NODES, HH, B, HB, F)
            .transpose(1, 3, 2, 4, 0, 5)
            .reshape(N_NODES, B, H_SH, W)
        )
        out[:, :, i * H_SH : (i + 1) * H_SH, :] = o
    return out


def kernel(spikes, masks, weights, src_idx, dst_idx, trace=False):
    spikes = np.asarray(spikes, dtype=np.float32)
    masks = np.asarray(masks, dtype=np.float32)
    weights = np.asarray(weights, dtype=np.float32)
    src = [int(x) for x in np.asarray(src_idx).ravel()]
    dst = [int(x) for x in np.asarray(dst_idx).ravel()]
    assert spikes.shape == (N_NODES, B, H, W)
    assert masks.shape == (N_NODES, H, W)
    assert weights.shape == (N_EDGES, H, W)
    assert len(src) == N_EDGES and len(dst) == N_EDGES

    nc = _trace_and_compile(src, dst)
    in_maps = make_in_maps(spikes, masks, weights, src, dst)
    res = run_bass_kernel_spmd(
        nc, in_maps, core_ids=list(range(N_CORES)), trace=trace
    )
    out = assemble_out(res.results)

    if trace:
        kernel.last_exec_time_ns = res.exec_time_ns
        kernel.last_results = res
    return out
